# revision 1
# baseline (speedup 1.0000x reference)
"""MoE model (64 experts, top-24 routing) on 8 Trainium2 NeuronCores.

Strategy: data-parallel shard of the 8192-token batch (1024 tokens/core).
Each core:
  - computes gate logits in exact fp32 (top-k selection fidelity),
  - top-24 masked-softmax routing weights via DVE max8/match_replace,
  - runs all 64 expert MLPs densely in float32r (TF32-like, full PE rate),
    streaming expert weights from HBM,
  - folds routing weights into the relu'd hidden activations (so layer-2
    matmuls accumulate the routing-weighted expert sum directly in PSUM
    across all 64 experts),
  - expert biases: b1 fused into the ReLU activation (per-partition bias),
    b2 applied as routing_weights @ b2 matmul opening the PSUM accumulation
    (softmax weights sum to 1 over selected experts, 0 elsewhere).

Layout notes (per core):
  xT   [1024(i), 1024(b)] : x shard transposed host-side
  L1:  h1T  [128h, 512b] = w1_chunk[128i,128h].T @ xT_chunk[128i,512b]
  s1s  = relu(h1T + b1) * wroute[b, e]   (broadcast along h)
  L2:  h2T  [128o, 512b] += w2_chunk[128h,128o].T @ s1s_chunk[128h,512b]
  out  = transpose(h2T) per 128x128 block at the end.
"""

import sys
import types

import numpy as np

import concourse.bass as bass
import concourse.tile as tile
import concourse.mybir as mybir
from concourse import bacc, bass_utils, masks

# bass_utils imports antenv.axon_hooks when BASS_TRACE=1; some images lack it.
# Provide a best-effort shim so tracing degrades instead of crashing.
try:
    import antenv.axon_hooks  # noqa: F401
except ImportError:
    try:
        import contextlib
        import ctypes

        def _make_hook():
            try:
                lib = ctypes.CDLL("/opt/axon/libaxon_pjrt.so")
            except OSError:
                return None
            if not hasattr(lib, "axon_start_nrt_profile"):
                return None
            lib.axon_start_nrt_profile.argtypes = [
                ctypes.POINTER(ctypes.c_int64), ctypes.c_size_t]
            lib.axon_start_nrt_profile.restype = ctypes.c_int64
            lib.axon_stop_nrt_profile.argtypes = [ctypes.c_char_p]
            lib.axon_stop_nrt_profile.restype = ctypes.c_int64

            @contextlib.contextmanager
            def _hook(output_dir, device_ids):
                import jax
                jax.devices()
                if device_ids:
                    ids = (ctypes.c_int64 * len(device_ids))(*device_ids)
                    rc = lib.axon_start_nrt_profile(ids, len(device_ids))
                else:
                    rc = lib.axon_start_nrt_profile(None, 0)
                if rc != 0:
                    raise RuntimeError(f"axon_start_nrt_profile rc={rc}")
                try:
                    yield
                finally:
                    lib.axon_stop_nrt_profile(str(output_dir).encode())

            return _hook

        _mod = types.ModuleType("antenv.axon_hooks")
        _mod.get_axon_ntff_profile_hook = _make_hook
        _mod.set_axon_ntff_profile_hook = lambda h: None
        sys.modules["antenv.axon_hooks"] = _mod
    except Exception:
        pass

F32 = mybir.dt.float32
F32R = mybir.dt.float32r
AF = mybir.ActivationFunctionType
ALU = mybir.AluOpType
AX = mybir.AxisListType

NCORES = 8
B = 8192
D = 1024          # input dim
H = 256           # hidden dim
O = 256           # output dim
NE = 64           # experts
TOPK = 24
BS = B // NCORES  # tokens per core (1024)
NBT = BS // 128   # b-tiles per core (8)
NG = BS // 512    # 512-token groups per core (2)
KC = D // 128     # contraction chunks for layer 1 (8)
HC = H // 128     # contraction chunks for layer 2 (2)
OC = O // 128     # output chunks (2)

_CACHE = {}


def _build():
    nc = bacc.Bacc("TRN2", target_bir_lowering=False, debug=False,
                   num_devices=NCORES)

    xt_d = nc.dram_tensor("xt", (D, BS), F32, kind="ExternalInput").ap()
    gw_d = nc.dram_tensor("gw", (D, NE), F32, kind="ExternalInput").ap()
    gb_d = nc.dram_tensor("gb", (NE, 1), F32, kind="ExternalInput").ap()
    w1_d = nc.dram_tensor("w1", (NE, D, H), F32R, kind="ExternalInput").ap()
    b1_d = nc.dram_tensor("b1p", (128, HC * NE), F32, kind="ExternalInput").ap()
    w2_d = nc.dram_tensor("w2", (NE, H, O), F32R, kind="ExternalInput").ap()
    b2_d = nc.dram_tensor("b2", (NE, O), F32, kind="ExternalInput").ap()
    out_d = nc.dram_tensor("out", (BS, O), F32, kind="ExternalOutput").ap()

    with tile.TileContext(nc) as tc:
        with tc.tile_pool(name="res", bufs=1) as res, \
             tc.tile_pool(name="w1p", bufs=3) as w1p, \
             tc.tile_pool(name="w2p", bufs=3) as w2p, \
             tc.tile_pool(name="s1p", bufs=3) as s1p, \
             tc.tile_pool(name="s1sp", bufs=3) as s1sp, \
             tc.tile_pool(name="wbp", bufs=3) as wbp, \
             tc.tile_pool(name="rt", bufs=2) as rt, \
             tc.tile_pool(name="ph1p", bufs=4, space="PSUM") as ph1p, \
             tc.tile_pool(name="ph2p", bufs=1, space="PSUM") as ph2p:

            # ---------------- resident loads ----------------
            # DMAs serialize on the Sync sequencer: emit gate inputs first and
            # chunk the x transfers so gate/L1 matmuls stream with the DMAs.
            gw_sb = res.tile([128, KC, NE], F32)
            nc.sync.dma_start(gw_sb[:], gw_d.rearrange("(c p) n -> p c n", p=128))
            gb_sb = res.tile([NE, 1], F32)
            nc.sync.dma_start(gb_sb[:], gb_d[:])

            def load_expert(e):
                w1_t = w1p.tile([128, KC, H], F32R, tag="w1", name=f"w1_{e}")
                nc.sync.dma_start(
                    w1_t[:], w1_d[e].rearrange("(c p) h -> p c h", p=128))
                w2_t = w2p.tile([128, HC, O], F32R, tag="w2", name=f"w2_{e}")
                nc.sync.dma_start(
                    w2_t[:], w2_d[e].rearrange("(c p) o -> p c o", p=128))
                return w1_t, w2_t

            xt_f = res.tile([128, KC, BS], F32)       # gate moving operand
            for ic in range(KC):
                nc.sync.dma_start(xt_f[:, ic, :],
                                  xt_d[ic * 128:(ic + 1) * 128, :])
            preload = {0: load_expert(0), 1: load_expert(1)}
            xt_r = res.tile([128, KC, BS], F32R)      # L1 moving operand
            for ic in range(KC):
                nc.sync.dma_start(
                    xt_r[:, ic, :],
                    xt_d.bitcast(F32R)[ic * 128:(ic + 1) * 128, :])
            b1_sb = res.tile([128, HC * NE], F32)
            nc.sync.dma_start(b1_sb[:], b1_d[:])
            b2_sb = res.tile([NE, O], F32)
            nc.sync.dma_start(b2_sb[:], b2_d[:])
            ident = res.tile([128, 128], F32)
            masks.make_identity(nc, ident[:])

            g_sb = res.tile([128, NBT, NE], F32)       # gate logits
            wroute = res.tile([128, NBT, NE], F32)     # routing weights
            wrouteT = res.tile([64, NBT, 128], F32)
            accT = res.tile([128, NG * OC, 512], F32)  # h2T evacuated
            acc = res.tile([128, NBT, O], F32)         # final [b, o]

            # h2T accumulator: 4 banks resident for the whole expert loop
            ph2acc = ph2p.tile([128, NG * OC, 512], F32, tag="ph2acc")

            # ---------------- gate logits (exact fp32) ----------------
            # gw stationary (LDW hides under the 4-cyc/row fp32 matmuls),
            # xt_f moving at N=512; output gateT [64n, 512b], bias folded
            # into the per-partition ACT evacuation, then PE-transposed.
            gT_sb = res.tile([64, NG, 512], F32)
            for g in range(NG):
                pgt = ph1p.tile([128, 512], F32, tag="ph1", name=f"pgt_{g}")
                for ic in range(KC):
                    nc.tensor.matmul(
                        pgt[0:NE, :],
                        gw_sb[:, ic, :],
                        xt_f[:, ic, g * 512:(g + 1) * 512],
                        start=(ic == 0), stop=(ic == KC - 1))
                nc.scalar.activation(gT_sb[:, g, :], pgt[0:NE, :],
                                     AF.Identity, bias=gb_sb[:], scale=1.0)
                # transpose this group's b-tiles immediately so the DVE
                # routing chain starts before the other group's gate matmuls
                for btl in range(4):
                    bt = g * 4 + btl
                    ptg = ph1p.tile([128, 512], F32, tag="ph1",
                                    name=f"ptg_{bt}")
                    nc.tensor.transpose(
                        ptg[:, 0:NE],
                        gT_sb[:, g, btl * 128:(btl + 1) * 128],
                        ident[0:NE, 0:NE])
                    nc.scalar.copy(g_sb[:, bt, :], ptg[:, 0:NE])

            # ---------------- top-24 masked softmax ----------------
            for bt in range(NBT):
                g = g_sb[:, bt, :]
                m8 = rt.tile([128, 3, 8], F32, tag="m8")
                gwk = rt.tile([128, 3, NE], F32, tag="gwk")
                nc.vector.max(m8[:, 0, :], g)
                nc.vector.match_replace(gwk[:, 0, :], m8[:, 0, :], g, -1e30)
                nc.vector.max(m8[:, 1, :], gwk[:, 0, :])
                nc.vector.match_replace(gwk[:, 1, :], m8[:, 1, :], gwk[:, 0, :], -1e30)
                nc.vector.max(m8[:, 2, :], gwk[:, 1, :])
                nc.vector.match_replace(gwk[:, 2, :], m8[:, 2, :], gwk[:, 1, :], -1e30)
                maskt = rt.tile([128, NE], F32, tag="maskt")
                nc.vector.tensor_scalar(maskt[:], gwk[:, 2, :], -1e29, None,
                                        op0=ALU.is_lt)
                negm1 = rt.tile([128, 1], F32, tag="negm1")
                nc.vector.tensor_scalar_mul(negm1[:], m8[:, 0, 0:1], -1.0)
                e_sb = rt.tile([128, NE], F32, tag="e_sb")
                nc.scalar.activation(e_sb[:], g, AF.Exp, bias=negm1[:], scale=1.0)
                em = rt.tile([128, NE], F32, tag="em")
                nc.vector.tensor_mul(em[:], e_sb[:], maskt[:])
                ssum = rt.tile([128, 1], F32, tag="ssum")
                nc.vector.reduce_sum(ssum[:], em[:], axis=AX.X)
                rsum = rt.tile([128, 1], F32, tag="rsum")
                nc.vector.reciprocal(rsum[:], ssum[:])
                nc.vector.tensor_scalar_mul(wroute[:, bt, :], em[:], rsum[:])

            # wrouteT transposes + b2 bias matmuls: emitted after L1+relu of
            # expert 0 so the PE covers the routing chain's tail.
            def emit_route_t_and_bias():
                for bt in range(NBT):
                    ptr_ = ph1p.tile([128, 512], F32, tag="ph1",
                                     name=f"ptr_{bt}")
                    nc.tensor.transpose(ptr_[0:64, 0:128], wroute[:, bt, :],
                                        ident[:])
                    nc.scalar.copy(wrouteT[:, bt, :], ptr_[0:64, 0:128])
                for g in range(NG):
                    for oc in range(OC):
                        nc.tensor.matmul(
                            ph2acc[:, g * OC + oc, :],
                            b2_sb[:, oc * 128:(oc + 1) * 128],
                            wrouteT[:, g * 4:(g + 1) * 4, :],
                            start=True, stop=False, skip_group_check=True)

            # ---------------- dense expert loop (software-pipelined) ------
            def emit_l1(e, w1_t):
                # g innermost: one stationary load (w1 chunk) feeds both
                # 512-token groups -> half the LDWEIGHTS traffic
                ph1 = [[ph1p.tile([128, 512], F32, tag="ph1",
                                  name=f"ph1_{e}_{g}_{i}")
                        for i in range(HC)] for g in range(NG)]
                for hc in range(HC):
                    for ic in range(KC):
                        for g in range(NG):
                            nc.tensor.matmul(
                                ph1[g][hc][:],
                                w1_t[:, ic, hc * 128:(hc + 1) * 128],
                                xt_r[:, ic, g * 512:(g + 1) * 512],
                                start=(ic == 0), stop=(ic == KC - 1))
                return ph1

            def emit_relu(e, ph1):
                s1 = []
                for g in range(NG):
                    s1_g = s1p.tile([128, HC, 512], F32, tag="s1",
                                    name=f"s1_{e}_{g}")
                    s1.append(s1_g)
                    for hc in range(HC):
                        nc.scalar.activation(
                            s1_g[:, hc, :], ph1[g][hc][:], AF.Relu,
                            bias=b1_sb[:, hc * NE + e: hc * NE + e + 1],
                            scale=1.0)
                return s1

            def emit_scale(e, s1):
                s1s = []
                for g in range(NG):
                    wb0 = wbp.tile([1, 512], F32, tag="wb0", name=f"wb0_{e}_{g}")
                    nc.sync.dma_start(wb0[:], wrouteT[e:e + 1, g * 4:(g + 1) * 4, :])
                    wb = wbp.tile([128, 512], F32, tag="wb", name=f"wb_{e}_{g}")
                    nc.gpsimd.partition_broadcast(wb[:], wb0[:])
                    s1s_g = s1sp.tile([128, HC, 512], F32R, tag="s1s",
                                      name=f"s1s_{e}_{g}")
                    s1s.append(s1s_g)
                    for hc in range(HC):
                        nc.vector.tensor_tensor(
                            s1s_g[:, hc, :], s1[g][:, hc, :], wb[:],
                            op=ALU.mult)
                return s1s

            def emit_l2(e, w2_t, s1s, last):
                for hc in range(HC):
                    for oc in range(OC):
                        for g in range(NG):
                            nc.tensor.matmul(
                                ph2acc[:, g * OC + oc, :],
                                w2_t[:, hc, oc * 128:(oc + 1) * 128],
                                s1s[g][:, hc, :],
                                start=False,
                                stop=(last and hc == HC - 1),
                                skip_group_check=True)

            w1_t0, w2_t0 = preload[0]
            ph1_0 = emit_l1(0, w1_t0)
            s1_0 = emit_relu(0, ph1_0)
            emit_route_t_and_bias()
            prev = (0, w2_t0, emit_scale(0, s1_0))
            for e in range(1, NE):
                w1_t, w2_t = preload[e] if e in preload else load_expert(e)
                ph1 = emit_l1(e, w1_t)
                s1s = emit_scale(e, emit_relu(e, ph1))
                emit_l2(prev[0], prev[1], prev[2], last=False)
                prev = (e, w2_t, s1s)
            emit_l2(prev[0], prev[1], prev[2], last=True)

            # ---------------- evacuate + transpose back + store ----------
            out_v = out_d.rearrange("(t p) o -> p t o", p=128)
            for g in range(NG):
                for oc in range(OC):
                    j = g * OC + oc
                    nc.vector.tensor_copy(accT[:, j, :], ph2acc[:, j, :])
                    for btl in range(4):
                        bt = g * 4 + btl
                        ptt = ph1p.tile([128, 512], F32, tag="ph1",
                                        name=f"ptt_{g}_{oc}_{btl}")
                        nc.tensor.transpose(
                            ptt[:, 0:128],
                            accT[:, j, btl * 128:(btl + 1) * 128],
                            ident[:])
                        nc.scalar.copy(acc[:, bt, oc * 128:(oc + 1) * 128],
                                       ptt[:, 0:128])
                    nc.sync.dma_start(
                        out_v[:, g * 4:(g + 1) * 4, oc * 128:(oc + 1) * 128],
                        acc[:, g * 4:(g + 1) * 4, oc * 128:(oc + 1) * 128])

    nc.compile()
    return nc


def _prep_host(gate_b, expert_b1):
    gb = np.ascontiguousarray(np.asarray(gate_b, dtype=np.float32).reshape(NE, 1))
    b1 = np.asarray(expert_b1, dtype=np.float32)          # [64, 256]
    b1p = np.ascontiguousarray(
        b1.reshape(NE, HC, 128).transpose(2, 1, 0).reshape(128, HC * NE))
    return gb, b1p


def kernel(x, gate_w, gate_b, expert_w1, expert_b1, expert_w2, expert_b2, k):
    assert int(k) == TOPK
    if "nc" not in _CACHE:
        _CACHE["nc"] = _build()
    nc = _CACHE["nc"]

    x = np.asarray(x, dtype=np.float32)
    gw = np.ascontiguousarray(np.asarray(gate_w, dtype=np.float32))
    w1 = np.ascontiguousarray(np.asarray(expert_w1, dtype=np.float32))
    w2 = np.ascontiguousarray(np.asarray(expert_w2, dtype=np.float32))
    b2 = np.ascontiguousarray(np.asarray(expert_b2, dtype=np.float32))
    gb, b1p = _prep_host(gate_b, expert_b1)

    in_maps = []
    for c in range(NCORES):
        xt = np.ascontiguousarray(x[c * BS:(c + 1) * BS].T)
        in_maps.append({"xt": xt, "gw": gw, "gb": gb, "w1": w1, "b1p": b1p,
                        "w2": w2, "b2": b2})

    r = bass_utils.run_bass_kernel_spmd(nc, in_maps, core_ids=list(range(NCORES)))
    _CACHE["last_result"] = r
    return np.concatenate([m["out"] for m in r.results], axis=0)

